# revision 1
# baseline (speedup 1.0000x reference)
"""Causal multihead self-attention with RoPE on 8 TRN2 NeuronCores.

Problem: B=2, S=2048, D=1024, H=16 heads, d_k=64, causal, RoPE theta=10000.

Sharding (Megatron-style, per hint): core c = 4*b + g handles batch b and the
4 heads [4g, 4g+4): Wq/Wk/Wv column-parallel (sliced rows of W since torch
computes x @ W.T), Wo row-parallel; each core emits a partial [S, D] output
and the host sums the 4 partials per batch.

Device kernel (per core); matmul operands in bf16 (fp32 PSUM accumulation,
fast weight loads), everything else fp32:
  A) qT/kT = W' @ x.T in a d-on-partition layout ([128, 2 chunks, S]); the
     d_k dims are permuted (folded into the weights on the host) so chunk 0
     holds the even RoPE lanes (x1) and chunk 1 the odd lanes (x2) for all 4
     heads; RoPE is then 6 full-width DVE ops per (tensor, qtile) against
     host-built cos/sin tables, and a DMA repack rearranges rows into a
     per-head-contiguous layout so each head's scores need ONE K=64 matmul.
     v is computed in natural [S, d] layout and packed as per-head
     [v | ones] / [ones | v] (the ones columns produce the softmax sums for
     free during the attention*V matmul, landing on the complementary lanes).
  B) Per (qtile, head pair): scoresT tiles [128 kpos, 2, 512 q] via PE
     (row-group packed, 2 heads concurrently), exp on ACT (no max
     subtraction needed: |scores| <= ~40), causal masking of
     diagonal-crossing tiles via a DVE multiply with host-built 0/1 tiles
     (fully-masked leading columns skipped), then AV accumulation in PSUM.
     Softmax normalization: 1/sums = exp(-ln(sums)) on ACT (both fns live in
     one activation-table set), then one DVE mul (psum x sbuf operands may
     use different base partitions) into the yT activation layout.
  C) partial = yT.T @ Wo_slice.T via PE, staged to SBUF, DMA to DRAM.

On top of the original schedule, three measured wins (~197 -> ~194-196us):
  - 90 dependency-free warmup matmuls during the initial DMA wait plus
    "keepwarm" dummy matmuls at pipeline-bubble sites (pair boundaries,
    RoPE/repack seams): the PE HAM activity monitor re-throttles the PE
    clock 2.4 -> 1.2 GHz after idle-ish 3.4us windows, and short ~1us
    bubbles otherwise poison several microseconds of matmuls each.
  - Diagonal-crossing score matmuls stream only the live q columns
    (f0 trim), saving ~6k PE cycles.
Rejected after measurement (all slower on HW): interleaving phase A(t+1)
into phase B(t) via emission weaving (in-order engine queues turn any
not-yet-ready dependency into a head-of-line stall; 212-218us), merged
2-ACT softmax normalization with a single av pair-tile (PSUM WAR stalls at
pair boundaries), multi-ring need-ordered DMA schedules (per-packet round
robin steals bandwidth from the critical ring; 208-233us).

Measured on 8 axon-attached TRN2 cores: ~194-196us HW exec, max rel err
3.7e-3 vs fp64-ish CPU reference (bf16-dominated; set USE_BF16=False for a
float32r build: ~380us, 2.2e-4).
"""
import sys

sys.path.insert(0, "/opt/trn_rl_repo")

import numpy as np

import concourse.bacc as bacc
import concourse.hw_specs as hw_specs
import concourse.tile as tile
from concourse import mybir
from concourse.bass_utils import run_bass_kernel_spmd

# Keep Exp and Ln in one activation-table set: hide them from every other set
# so bacc's table-load pass picks natural_log_exp_and_others for both instead
# of thrashing between exp_and_others and natural_log (~2.7us per reload).
_orig_act_tables = hw_specs.get_activation_tables


def _patched_act_tables(arch):
    _E = mybir.ActivationFunctionType.Exp
    _L = mybir.ActivationFunctionType.Ln
    out = {}
    for name, fns in _orig_act_tables(arch).items():
        if name != "natural_log_exp_and_others":
            fns = fns - {_E, _L}
        out[name] = fns
    return out


bacc.get_activation_tables = _patched_act_tables

F32 = mybir.dt.float32
F32R = mybir.dt.float32r
BF16 = mybir.dt.bfloat16
USE_BF16 = True
MM_DT = BF16 if USE_BF16 else F32R
EXP = mybir.ActivationFunctionType.Exp
LN = mybir.ActivationFunctionType.Ln
MUL = mybir.AluOpType.mult
ADD = mybir.AluOpType.add
SUB = mybir.AluOpType.subtract

B, S, D = 2, 2048, 1024
H, DK = 16, 64          # global heads, head dim
HL = 4                  # heads per core
GD = HL * DK            # local width 256
T = S // 512            # 4 q-tiles of 512
C = S // 128            # 16 kpos chunks of 128
DCH = D // 128          # 8 contraction chunks
THETA = 10000.0

_cache = {}


def _build_kernel():
    nc = bacc.Bacc("TRN2", target_bir_lowering=False, debug=False, num_devices=8)

    xT = nc.declare_dram_parameter("xT", [D, S], MM_DT, isOutput=False)
    wq = nc.declare_dram_parameter("wq", [D, GD], MM_DT, isOutput=False)
    wk = nc.declare_dram_parameter("wk", [D, GD], MM_DT, isOutput=False)
    wv = nc.declare_dram_parameter("wv", [D, GD], MM_DT, isOutput=False)
    wo = nc.declare_dram_parameter("wo", [GD, D], MM_DT, isOutput=False)
    ccd = nc.declare_dram_parameter("cc", [32, S], F32, isOutput=False)
    ssd = nc.declare_dram_parameter("ss", [32, S], F32, isOutput=False)
    mskd = nc.declare_dram_parameter("msk", [128, 4, 2, 512], MM_DT,
                                     isOutput=False)
    out = nc.declare_dram_parameter("out", [S, D], F32, isOutput=True)

    with tile.TileContext(nc) as tc:
        with (
            tc.tile_pool(name="consts", bufs=1) as consts,
            tc.tile_pool(name="persist", bufs=1) as persist,
            tc.tile_pool(name="xtp", bufs=3) as xtp,
            tc.tile_pool(name="rtmp", bufs=8) as rtmp,
            tc.tile_pool(name="rop", bufs=3) as rop,
            tc.tile_pool(name="ep", bufs=8) as ep,
            tc.tile_pool(name="rp", bufs=6) as rp,
            tc.tile_pool(name="osb", bufs=3) as osb,
            tc.tile_pool(name="pp", bufs=1, space="PSUM") as pp,
            tc.tile_pool(name="scp", bufs=2, space="PSUM") as scp,
            tc.tile_pool(name="avp", bufs=3, space="PSUM") as avp,
        ):
            # ---- constants (emission order = DMA priority: the first
            # projection matmuls need xt(t=0) + wq/wk before anything else) ----
            wq_t = consts.tile([128, DCH, GD], MM_DT, tag="wq")
            wk_t = consts.tile([128, DCH, GD], MM_DT, tag="wk")
            wv_t = consts.tile([128, DCH, GD], MM_DT, tag="wv")
            wo_t = consts.tile([128, 2, D], MM_DT, tag="wo")
            cc_t = consts.tile([128, S], F32, tag="cc")
            ss_t = consts.tile([128, S], F32, tag="ss")

            xts = {}
            xts[0] = xtp.tile([128, DCH, 512], MM_DT, tag="xt", name="xt0")
            xT_v = xT.rearrange("(c p) s -> p c s", p=128)
            wq_v = wq.rearrange("(c p) g -> p c g", p=128)
            for rb in range(4):
                nc.sync.dma_start(out=cc_t[32 * rb:32 * (rb + 1), :],
                                  in_=ccd[:])
                nc.sync.dma_start(out=ss_t[32 * rb:32 * (rb + 1), :],
                                  in_=ssd[:])
            for dd in range(DCH):
                nc.sync.dma_start(out=xts[0][:, dd, :], in_=xT_v[:, dd, 0:512])
                nc.sync.dma_start(out=wq_t[:, dd, :], in_=wq_v[:, dd, :])
            nc.sync.dma_start(
                out=wk_t[:], in_=wk.rearrange("(c p) g -> p c g", p=128))
            nc.sync.dma_start(
                out=wv_t[:], in_=wv.rearrange("(c p) g -> p c g", p=128))
            msk_t = consts.tile([128, 4, 2, 512], MM_DT, tag="msk")
            nc.sync.dma_start(out=msk_t[:], in_=mskd[:])

            # PE warmup during the initial DMA wait + a helper that keeps
            # the PE HAM activity monitor fed across pipeline bubbles (an
            # idle-ish 3.4us window re-throttles the PE clock to 1.2 GHz
            # for several microseconds)
            warm_f = consts.tile([128, 128], F32, tag="warmf")
            nc.vector.memset(warm_f[:], 0.03125)
            warm = consts.tile([128, 128], MM_DT, tag="warm")
            nc.vector.tensor_copy(warm[:], warm_f[:])
            wps = pp.tile([128, 512], F32, tag="pp", name="warmps")
            for i in range(90):
                nc.tensor.matmul(wps[:, 0:128], lhsT=warm[:], rhs=warm[:],
                                 start=(i == 0), stop=(i == 89))

            kw_n = [0]

            def keepwarm(n=2):
                kw_n[0] += 1
                wt = pp.tile([128, 512], F32, tag="pp",
                             name=f"kw{kw_n[0]}")
                for i in range(n):
                    nc.tensor.matmul(wt[:, 0:128], lhsT=warm[:],
                                     rhs=warm[:], start=True, stop=True)

            ones_f = consts.tile([128, 2, DK], F32, tag="onesf")
            nc.vector.memset(ones_f[:], 1.0)
            ones = consts.tile([128, 2, DK], MM_DT, tag="ones")
            nc.vector.tensor_copy(ones[:], ones_f[:])

            # persistent activations
            qT = persist.tile([128, 2, S], MM_DT, tag="qT")
            kT = persist.tile([128, 2, S], MM_DT, tag="kT")
            yT = persist.tile([128, 2, S], MM_DT, tag="yT")
            v_ext = persist.tile([128, C, HL, 2 * DK], MM_DT, tag="vext")

            # ones halves of v_ext: even heads [64:128], odd heads [0:64]
            for c in range(C):
                for par, sl in ((0, slice(DK, 2 * DK)), (1, slice(0, DK))):
                    nc.vector.tensor_copy(v_ext[:, c, par::2, sl], ones[:])

            # ---- Phase A: projections + RoPE + v packing ----
            for t in range(T):
                qs = slice(512 * t, 512 * (t + 1))
                if t not in xts:
                    xts[t] = xtp.tile([128, DCH, 512], MM_DT, tag="xt",
                                      name=f"xt{t}")
                    nc.sync.dma_start(
                        out=xts[t][:],
                        in_=xT.rearrange("(c p) s -> p c s", p=128)[:, :, qs],
                    )
                xt = xts[t]
                for w_t, dst in ((wq_t, qT), (wk_t, kT)):
                    ccs, sss = cc_t[:, qs], ss_t[:, qs]
                    t1 = rtmp.tile([128, 512], F32, tag="rt")
                    t2 = rtmp.tile([128, 512], F32, tag="rt")
                    t3 = rtmp.tile([128, 512], F32, tag="rt")
                    t4 = rtmp.tile([128, 512], F32, tag="rt")
                    ro = rop.tile([128, 2, 512], MM_DT, tag="ro")
                    # two-pass RoPE: hold only one projection psum at a time
                    for oc in range(2):
                        ps = avp.tile([128, 512], F32, tag="av",
                                      name=f"ps_{t}_{oc}")
                        for d in range(DCH):
                            nc.tensor.matmul(
                                ps[:],
                                lhsT=w_t[:, d, 128 * oc:128 * (oc + 1)],
                                rhs=xt[:, d, :],
                                start=(d == 0),
                                stop=(d == DCH - 1),
                            )
                        if oc == 0:
                            nc.vector.tensor_tensor(t1[:], ps[:], ccs, op=MUL)
                            nc.vector.tensor_tensor(t3[:], ps[:], sss, op=MUL)
                        else:
                            nc.vector.tensor_tensor(t2[:], ps[:], sss, op=MUL)
                            nc.vector.tensor_tensor(ro[:, 0, :], t1[:], t2[:],
                                                    op=SUB)
                            nc.vector.tensor_tensor(t4[:], ps[:], ccs, op=MUL)
                    nc.vector.tensor_tensor(ro[:, 1, :], t3[:], t4[:], op=ADD)
                    # repack to per-head-contiguous rows (half 0 first --
                    # ro[:, 0] is ready one DVE op earlier):
                    # dst rows 64*hp + 32*half + j, chunk oc = head pair
                    dma_eng = nc.gpsimd if dst is qT else nc.sync
                    for half in range(2):
                        for oc in range(2):
                            for hp in range(2):
                                sp = 32 * (2 * oc + hp)
                                dp = 64 * hp + 32 * half
                                dma_eng.dma_start(
                                    out=dst[dp:dp + 32, oc, qs],
                                    in_=ro[sp:sp + 32, half, :],
                                )
                    keepwarm(3)

                for s4 in range(4):
                    s = 4 * t + s4
                    psv = avp.tile([128, 512], F32, tag="av",
                                   name=f"psv_{t}_{s4}")
                    for d in range(DCH):
                        nc.tensor.matmul(
                            psv[:, :GD],
                            lhsT=xt[:, d, 128 * s4:128 * (s4 + 1)],
                            rhs=wv_t[:, d, :],
                            start=(d == 0),
                            stop=(d == DCH - 1),
                        )
                    pv = psv[:, :GD].rearrange("p (h e) -> p h e", e=DK)
                    for par, sl in ((0, slice(0, DK)), (1, slice(DK, 2 * DK))):
                        nc.vector.tensor_copy(
                            v_ext[:, s, par::2, sl], pv[:, par::2, :])
                    keepwarm(2)

            # ---- Phase B: attention (with phase C interleaved per qtile) ----
            wo_loaded = False
            for t in range(T):
                qs = slice(512 * t, 512 * (t + 1))
                for pair in range(2):
                    heads = (2 * pair, 2 * pair + 1)
                    av_ps = {}
                    for h in heads:
                        av_ps[h] = avp.tile([128, 512], F32, tag="av",
                                            name=f"av_{t}_{h}")
                    nck = 4 * t + 4  # eligible kpos chunks
                    # software-pipelined emission: AV runs one chunk
                    # behind QK/exp so the in-order PE queue never blocks
                    # the next chunk's score matmuls behind an AV that is
                    # still waiting for its exp
                    pend = []  # [(c, e, f0)] awaiting AV, 2-chunk skew
                    for c in range(nck):
                        ks = slice(128 * c, 128 * (c + 1))
                        # columns f < 128*j of a diagonal-crossing tile are
                        # fully masked -- skip them in the score matmuls too
                        j = c - 4 * t
                        f0 = 128 * j if (0 < j < 4 and c > 0) else 0
                        sc = scp.tile([128, 2, 512], F32, tag="sc")
                        for hp in range(2):
                            rows = slice(64 * hp, 64 * hp + 64)
                            nc.tensor.matmul(
                                sc[:, hp, f0:],
                                lhsT=kT[rows, pair, ks],
                                rhs=qT[rows, pair,
                                       512 * t + f0:512 * (t + 1)],
                                start=True, stop=True,
                                tile_position=(64 * hp, 0),
                            )
                        e = ep.tile([128, 2, 512], MM_DT, tag="e")
                        nc.scalar.activation(e[:, :, f0:], sc[:, :, f0:], EXP)
                        if c >= 4 * t:  # diagonal-crossing tile
                            nc.vector.tensor_tensor(
                                e[:, :, f0:], e[:, :, f0:],
                                msk_t[:, j, :, f0:], op=MUL)
                        pend.append((c, e, f0))
                        if len(pend) > 2:
                            pc, pe_, pf0 = pend.pop(0)
                            for hp, h in enumerate(heads):
                                nc.tensor.matmul(
                                    av_ps[h][:, pf0:],
                                    lhsT=v_ext[:, pc, h, :],
                                    rhs=pe_[:, hp, pf0:],
                                    start=(pc == 0),
                                    stop=False,
                                )
                    while pend:
                        pc, pe_, pf0 = pend.pop(0)
                        for hp, h in enumerate(heads):
                            nc.tensor.matmul(
                                av_ps[h][:, pf0:],
                                lhsT=v_ext[:, pc, h, :],
                                rhs=pe_[:, hp, pf0:],
                                start=(pc == 0),
                                stop=(not pend),
                            )
                    for h in heads:
                        # sums rows / out rows by head parity
                        if h % 2 == 0:
                            srows, orows = slice(64, 128), slice(0, 64)
                        else:
                            srows, orows = slice(0, 64), slice(64, 128)
                        r1 = rp.tile([128, 512], F32, tag="rr")
                        r2 = rp.tile([128, 512], F32, tag="rr")
                        nc.scalar.activation(r1[srows], av_ps[h][srows], LN)
                        nc.scalar.activation(r2[srows], r1[srows], EXP,
                                             scale=-1.0)
                        # psum + sbuf operands may use different base
                        # partitions (walrus only requires equality for
                        # SB+SB pairs)
                        nc.vector.tensor_tensor(
                            yT[orows, h // 2, qs],
                            av_ps[h][orows], r2[srows], op=MUL)
                    keepwarm(7)

                # ---- Phase C slice for this qtile ----
                if not wo_loaded:
                    nc.sync.dma_start(
                        out=wo_t[:],
                        in_=wo.rearrange("(c p) d -> p c d", p=128))
                    wo_loaded = True
                for s_ in range(4 * t, 4 * t + 4):
                    ssl = slice(128 * s_, 128 * (s_ + 1))
                    for n in range(2):
                        nsl = slice(512 * n, 512 * (n + 1))
                        if t == 3:
                            # attention done by now: the av slots are free,
                            # giving the final-output chain 3-deep pipelining
                            po = avp.tile([128, 512], F32, tag="av",
                                          name=f"po3_{s_}_{n}")
                        else:
                            po = pp.tile([128, 512], F32, tag="pp")
                        for ldc in range(2):
                            nc.tensor.matmul(
                                po[:],
                                lhsT=yT[:, ldc, ssl],
                                rhs=wo_t[:, ldc, nsl],
                                start=(ldc == 0),
                                stop=(ldc == 1),
                            )
                        ob = osb.tile([128, 512], F32, tag="ob")
                        if t == 3:
                            nc.scalar.copy(ob[:], po[:])
                        else:
                            nc.vector.tensor_copy(ob[:], po[:])
                        nc.sync.dma_start(out=out[ssl, nsl], in_=ob[:])
                    if t == 3:
                        keepwarm(3)

    nc.compile()
    return nc


def _host_prep(x, token_positions, Wq, Wk, Wv, Wo):
    # d_k permutation folded into Wq/Wk.  Projection-output row n (0..255):
    # chunk oc = n//128 (all x1 lanes in chunk 0, x2 in chunk 1 for RoPE),
    # head h = (n%128)//32, freq j = n%32 -> orig row 64h + 2j + oc.
    # (The post-RoPE repack DMA then rearranges rows per-head-contiguous.)
    n = np.arange(GD)
    chunk = n // 128
    hh = (n % 128) // 32
    jj = n % 32
    perm = 64 * hh + 2 * jj + chunk

    pos = np.asarray(token_positions).astype(np.float64)
    inv_freq = THETA ** (-np.arange(0, DK, 2, dtype=np.float64) / DK)  # [32]
    ang = pos[:, None] * inv_freq[None, :]                             # [S, 32]
    cos = np.cos(ang).astype(np.float32)
    sin = np.sin(ang).astype(np.float32)
    # [32, S]: rows = freq j (replicated to 128 partitions on device)
    cc = np.ascontiguousarray(cos.T)
    ss = np.ascontiguousarray(sin.T)

    # causal mask tiles for diagonal-crossing chunks: msk[p, j, :, f] = 1.0
    # iff f >= p + 128*j (duplicated across the head-pair dim)
    pp_, ff_ = np.arange(128)[:, None], np.arange(512)[None, :]
    msk1 = np.stack([(ff_ >= pp_ + 128 * j) for j in range(4)], 0)  # [4,128,512]
    msk = np.repeat(msk1.transpose(1, 0, 2)[:, :, None, :], 2, axis=2)

    scale = 1.0 / np.sqrt(np.float32(DK))
    if USE_BF16:
        import ml_dtypes
        mmnp = ml_dtypes.bfloat16
    else:
        mmnp = np.float32
    in_maps = []
    for core in range(8):
        b, g = divmod(core, 4)
        gsl = slice(GD * g, GD * (g + 1))
        in_maps.append({
            "xT": np.ascontiguousarray(np.asarray(x[b], np.float32).T).astype(mmnp),
            "wq": np.ascontiguousarray(
                (np.asarray(Wq[gsl], np.float32) * scale)[perm].T.astype(mmnp)),
            "wk": np.ascontiguousarray(np.asarray(Wk[gsl], np.float32)[perm].T.astype(mmnp)),
            "wv": np.ascontiguousarray(np.asarray(Wv[gsl], np.float32).T.astype(mmnp)),
            "wo": np.ascontiguousarray(np.asarray(Wo[:, gsl], np.float32).T.astype(mmnp)),
            "cc": cc,
            "ss": ss,
            "msk": np.ascontiguousarray(msk).astype(mmnp),
        })
    return in_maps


def kernel(x, token_positions, Wq, Wk, Wv, Wo, _trace=False, _result=[None],
           _tmpdir=None):
    if "nc" not in _cache:
        _cache["nc"] = _build_kernel()
    nc = _cache["nc"]
    in_maps = _host_prep(x, token_positions, Wq, Wk, Wv, Wo)
    res = None
    for attempt in range(3):
        try:
            res = run_bass_kernel_spmd(
                nc, in_maps, core_ids=list(range(8)), trace=_trace,
                tmpdir=_tmpdir)
            break
        except Exception:
            # transient NRT_EXEC_UNIT_UNRECOVERABLE device hiccups resolve
            # on retry
            if attempt == 2:
                raise
    _result[0] = res
    outs = np.stack([r["out"] for r in res.results])  # [8, S, D]
    full = outs.reshape(B, 4, S, D).sum(axis=1, dtype=np.float32)
    return full

